# revision 1
# baseline (speedup 1.0000x reference)
"""Trainium2 Bass kernel for the Attention2 module.

Computation (per batch row b):
    att_h  = h[b] @ W_h.T + b_h                      # [A]
    dot    = tanh(p_att_feats[b] + att_h)            # [L, A]
    scores = dot @ W_a[0]  (+ b_a, dropped: softmax shift-invariant)
    scores = where(mask, -1e8, scores)
    w      = softmax(scores)                         # [L]
    out[b] = w @ att_feats[b]                        # [R]

Sharding: data-parallel over batch B=32 across 8 cores (4 rows/core).

Per-core mapping (L=2048 -> 16 chunks of 128 partitions):
  phase 0: att_h for the 4 local rows via PE (K=RNN on partitions),
           +b_h on DVE, partition-broadcast via K=1 ones-matmuls.
  phase A: p-tile [128(l), 512(a)]; DVE add of broadcast att_h; ACT tanh
           (in place); DVE multiply by broadcast W_a + free-dim
           reduce_sum -> scores column [128, 1].
  softmax: no max subtraction (|scores| <= ~23 so exp can't overflow);
           ACT exp, mask applied multiplicatively (keep in {0,1}) on
           DVE, row-sum on DVE; partition sum via a ones-vector PE
           matmul; reciprocal on DVE.  exp(-1e8) == 0 in the
           reference, identical to multiplying exp(s) by 0.
  phase B: out[b] = sum_l w[l] * att_feats[b,l,:] as PE matmuls:
           lhsT = w column [128(l), 1], rhs = f-tile [128(l), 512(r)],
           accumulated over the 16 l-chunks into PSUM [1, 512] x 2.
           float32r (full-rate fp32 matmul mode) on both operands.
  scale:   result * (1/Z) on DVE, DMA out.

Host-side prep is layout only: transposes of h/W_h (so the contraction
dim lands on partitions with unit-stride loads) and the boolean mask
converted to a float keep-mask in score layout.
"""

import sys

import ml_dtypes
import numpy as np

sys.path.insert(0, "/opt/trn_rl_repo")

import concourse.bass as bass  # noqa: E402
import concourse.tile as tile  # noqa: E402
from concourse import bacc, mybir  # noqa: E402
from concourse.bass_utils import run_bass_kernel_spmd  # noqa: E402

N_CORES = 8
B, L, RNN, A = 32, 2048, 1024, 512
BS = B // N_CORES

F32 = mybir.dt.float32
F32R = mybir.dt.float32r
BF16 = mybir.dt.bfloat16
MULT = mybir.AluOpType.mult
ADD = mybir.AluOpType.add
TANH = mybir.ActivationFunctionType.Tanh
EXP = mybir.ActivationFunctionType.Exp


KERNEL_VERSION = 8

import os  # noqa: E402

_FAST = os.environ.get("KERNEL_FAST", "0")


def build_program(bs=BS, ll=L, rnn=RNN, a=A, lgrp=4, use_f32r=True,
                  p_bufs=3, f_bufs=4, add_on_gpsimd=_FAST == "1",
                  reduce_on_act=_FAST in ("1", "2")):
    nch = ll // 128          # l-chunks of 128 partitions
    ng = nch // lgrp         # DMA groups (lgrp l-chunks per transfer)
    kch = rnn // 128         # contraction chunks for att_h
    nh = (rnn + 511) // 512  # 512-wide output halves of phase B
    rh = rnn // nh           # free width per output half

    fdt = F32R if use_f32r else F32
    nc = bacc.Bacc(None, target_bir_lowering=False)
    p = nc.dram_tensor("p", [bs, ll, a], BF16, kind="ExternalInput")
    f = nc.dram_tensor("f", [bs, ll, rnn], fdt, kind="ExternalInput")
    hT = nc.dram_tensor("hT", [rnn, bs], F32, kind="ExternalInput")
    whT = nc.dram_tensor("whT", [rnn, a], F32, kind="ExternalInput")
    bh = nc.dram_tensor("bh", [1, a], F32, kind="ExternalInput")
    wa = nc.dram_tensor("wa", [1, a], F32, kind="ExternalInput")
    keep = nc.dram_tensor("keep", [bs, 128, nch], F32, kind="ExternalInput")
    # unused input whose SHAPE encodes the kernel version: the compile
    # cache keys on the HLO signature (names/shapes), NOT the embedded
    # BIR — without this, a rebuilt kernel with unchanged I/O silently
    # re-runs the previously cached NEFF.
    _code = (1 if add_on_gpsimd else 0) + (2 if reduce_on_act else 0)
    vcode = KERNEL_VERSION if _code == 0 else KERNEL_VERSION * 4 + _code
    ver = nc.dram_tensor("ver", [vcode, 1], F32, kind="ExternalInput")
    out = nc.dram_tensor("out", [bs, rnn], F32, kind="ExternalOutput")

    pr = p[:, :, :].rearrange("b (n q) a -> b q n a", q=128)
    fr = f[:, :, :].rearrange("b (n q) r -> b q n r", q=128)
    hTr = hT[:, :].rearrange("(c q) b -> q c b", q=128)
    whTr = whT[:, :].rearrange("(c q) a -> q c a", q=128)
    keepr = keep[:, :, :].rearrange("b q n -> q b n")

    with tile.TileContext(nc) as tc:
        with (
            tc.tile_pool(name="singles", bufs=1) as singles,
            tc.tile_pool(name="ppool", bufs=p_bufs) as ppool,
            tc.tile_pool(name="fpool", bufs=f_bufs) as fpool,
            tc.tile_pool(name="sm", bufs=3) as smpool,
            tc.tile_pool(name="respool", bufs=2) as respool,
            tc.tile_pool(name="psacc", bufs=2, space="PSUM") as psacc,
            tc.tile_pool(name="pssmall", bufs=2, space="PSUM") as pssmall,
        ):
            # ---- constants ----
            hT_sb = singles.tile([128, kch, bs], F32)
            nc.sync.dma_start(out=hT_sb, in_=hTr)
            whT_sb = singles.tile([128, kch, a], F32)
            nc.sync.dma_start(out=whT_sb, in_=whTr)
            # plain single-row loads (partition 0)
            bh_row = singles.tile([1, a], F32)
            nc.sync.dma_start(out=bh_row, in_=bh[:, :])
            wa_row = singles.tile([1, a], F32)
            nc.sync.dma_start(out=wa_row, in_=wa[:, :])
            keep_sb = singles.tile([128, bs, nch], F32)
            nc.sync.dma_start(out=keep_sb, in_=keepr)
            ver_sb = singles.tile([vcode, 1], F32)
            nc.sync.dma_start(out=ver_sb, in_=ver[:, :])
            ones_sb = singles.tile([128, 1], F32)
            nc.vector.memset(ones_sb, 1.0)
            # ones row for K=1 partition-broadcast matmuls
            ones_row = singles.tile([1, 128], F32)
            nc.vector.memset(ones_row, 1.0)
            wa_sb = singles.tile([128, a], BF16)
            attb_bc = singles.tile([128, bs, a], BF16)

            # ---- phase 0: attb[b] = h[b] @ W_h.T + b_h, then broadcast
            # across all 128 partitions via a K=1 ones-matmul (avoids
            # irregular 0-stride broadcast DMAs entirely).
            with tc.tile_pool(name="ps0", bufs=1, space="PSUM") as ps0:
                wa_ps = ps0.tile([128, a], F32, tag="bc")
                nc.tensor.matmul(wa_ps, lhsT=ones_row, rhs=wa_row,
                                 start=True, stop=True)
                nc.scalar.copy(out=wa_sb, in_=wa_ps)
                for b in range(bs):
                    ah_ps = ps0.tile([1, a], F32, tag="ah", name=f"ah{b}")
                    for c in range(kch):
                        nc.tensor.matmul(ah_ps, lhsT=hT_sb[:, c, b:b + 1],
                                         rhs=whT_sb[:, c, :],
                                         start=(c == 0), stop=(c == kch - 1))
                    attb_row = smpool.tile([1, a], F32, tag="attbrow")
                    nc.vector.tensor_add(attb_row, ah_ps, bh_row)
                    bc_ps = ps0.tile([128, a], F32, tag="bc", name=f"bc{b}")
                    nc.tensor.matmul(bc_ps, lhsT=ones_row, rhs=attb_row,
                                     start=True, stop=True)
                    nc.scalar.copy(out=attb_bc[:, b, :], in_=bc_ps)

            for b in range(bs):
                # ---- phase A: scores[l] = W_a . tanh(p[l] + attb) ----
                scores = smpool.tile([128, nch], F32, tag="scores")
                for t in range(ng):
                    ptile = ppool.tile([128, lgrp, a], BF16, tag="p")
                    nc.sync.dma_start(
                        out=ptile, in_=pr[b, :, t * lgrp:(t + 1) * lgrp, :])
                    add_eng = nc.gpsimd if add_on_gpsimd else nc.vector
                    for j in range(lgrp):
                        add_eng.tensor_add(
                            ptile[:, j, :], ptile[:, j, :], attb_bc[:, b, :])
                    nc.scalar.activation(out=ptile, in_=ptile, func=TANH)
                    for j in range(lgrp):
                        i = t * lgrp + j
                        # multiply by W_a on DVE, free-dim sum via the
                        # reduce primitive (tensor_tensor_reduce crashes
                        # execution on this runtime)
                        nc.vector.tensor_mul(
                            ptile[:, j, :], ptile[:, j, :], wa_sb)
                        if reduce_on_act:
                            nc.scalar.activation(
                                out=ptile[:, j, :], in_=ptile[:, j, :],
                                func=mybir.ActivationFunctionType.Copy,
                                accum_out=scores[:, i:i + 1])
                        else:
                            nc.vector.reduce_sum(
                                scores[:, i:i + 1], ptile[:, j, :],
                                axis=mybir.AxisListType.X)

                # ---- softmax weights (no max subtraction needed) ----
                e_sb = smpool.tile([128, nch], F32, tag="e")
                nc.scalar.activation(out=e_sb, in_=scores, func=EXP)
                w_sb = smpool.tile([128, nch], fdt, tag="w")
                nc.vector.tensor_mul(w_sb, e_sb, keep_sb[:, b, :])
                zpart = smpool.tile([128, 1], F32, tag="zpart")
                nc.vector.reduce_sum(zpart, w_sb, axis=mybir.AxisListType.X)
                z_ps = pssmall.tile([1, 1], F32, tag="zps")
                nc.tensor.matmul(z_ps, lhsT=ones_sb, rhs=zpart,
                                 start=True, stop=True)
                zinv = smpool.tile([1, 1], F32, tag="zinv")
                nc.vector.reciprocal(zinv, z_ps)

                # ---- phase B: out[b] = (w/Z) @ att_feats[b] ----
                rps = [psacc.tile([1, rh], F32, tag=f"r{hh}", name=f"rps{hh}")
                       for hh in range(nh)]
                for t in range(ng):
                    ftile = fpool.tile([128, lgrp, rnn], fdt, tag="f")
                    nc.sync.dma_start(
                        out=ftile, in_=fr[b, :, t * lgrp:(t + 1) * lgrp, :])
                    for j in range(lgrp):
                        i = t * lgrp + j
                        lhs = w_sb[:, i:i + 1]
                        for hh in range(nh):
                            nc.tensor.matmul(
                                rps[hh], lhsT=lhs,
                                rhs=ftile[:, j, hh * rh:(hh + 1) * rh],
                                start=(i == 0), stop=(i == nch - 1))
                res = respool.tile([1, rnn], F32, tag="res")
                for hh in range(nh):
                    nc.vector.tensor_scalar_mul(
                        res[:, hh * rh:(hh + 1) * rh], rps[hh], zinv)
                nc.sync.dma_start(out=out[b:b + 1, :], in_=res)
    nc.finalize()
    return nc


_PROG = None


def _get_program():
    global _PROG
    if _PROG is None:
        _PROG = build_program()
    return _PROG


def make_in_maps(h, att_feats, p_att_feats, mask, W_h, b_h, W_a):
    h = np.ascontiguousarray(np.asarray(h, dtype=np.float32))
    att_feats = np.asarray(att_feats, dtype=np.float32)
    p_att_feats = np.asarray(p_att_feats, dtype=np.float32)
    mask = np.asarray(mask)

    hT = np.ascontiguousarray(h.T)                                 # [RNN, B]
    whT = np.ascontiguousarray(np.asarray(W_h, np.float32).T)      # [RNN, A]
    bh = np.ascontiguousarray(np.asarray(b_h, np.float32).reshape(1, A))
    wa = np.ascontiguousarray(np.asarray(W_a, np.float32).reshape(1, A))
    # keep[b, p, i] = 1 - mask[b, i*128 + p]  (score-layout keep mask)
    keep = np.ascontiguousarray(
        (~mask).astype(np.float32).reshape(B, L // 128, 128).transpose(0, 2, 1))

    ver = np.zeros((KERNEL_VERSION, 1), np.float32)
    in_maps = []
    for c in range(N_CORES):
        s = slice(c * BS, (c + 1) * BS)
        in_maps.append({
            "p": np.ascontiguousarray(
                p_att_feats[s].astype(ml_dtypes.bfloat16)),
            "f": np.ascontiguousarray(att_feats[s]),
            "hT": np.ascontiguousarray(hT[:, s]),
            "whT": whT,
            "bh": bh,
            "wa": wa,
            "keep": np.ascontiguousarray(keep[s]),
            "ver": ver,
        })
    return in_maps


def run_sharded(inputs, trace=False, **kwargs):
    nc = _get_program()
    in_maps = make_in_maps(
        inputs["h"], inputs["att_feats"], inputs["p_att_feats"],
        inputs["mask"], inputs["W_h"], inputs["b_h"], inputs["W_a"])
    return run_bass_kernel_spmd(nc, in_maps, core_ids=list(range(N_CORES)),
                                trace=trace, **kwargs)


def kernel(h, att_feats, p_att_feats, mask, W_h, b_h, W_a, b_a):
    res = run_sharded({
        "h": h, "att_feats": att_feats, "p_att_feats": p_att_feats,
        "mask": mask, "W_h": W_h, "b_h": b_h, "W_a": W_a, "b_a": b_a})
    return np.concatenate([res.results[c]["out"] for c in range(N_CORES)],
                          axis=0).astype(np.float32)



# revision 5
# speedup vs baseline: 3.2031x; 3.2031x over previous
"""Trainium2 Bass kernel for the Attention2 module (sparse attention).

Computation (per batch row b):
    att_h  = h[b] @ W_h.T + b_h                      # [A]
    dot    = tanh(p_att_feats[b] + att_h)            # [L, A]
    scores = dot @ W_a[0]  (+ b_a, dropped: softmax shift-invariant)
    scores = where(mask, -1e8, scores)
    w      = softmax(scores)                         # [L]
    out[b] = w @ att_feats[b]                        # [R]

Sharding: data-parallel over batch B=32 across 8 cores (4 rows/core).

Key optimizations vs the dense baseline:
  * mask-compaction on host: masked positions get softmax weight exactly
    0 in the reference (exp(-1e8) == 0), so their p/f rows never need to
    reach the device.  Rows are gathered to the front and padded to a
    multiple of 128.  Padding p-rows are filled with -sign(W_a)*20 so
    tanh saturates to -sign(W_a) and the padded score is exactly
    -sum|W_a| ~= -11.3 -- its softmax contribution (~e-11 vs real scores
    ~e^0) is < 1e-5 relative, and the padded f-rows are zero anyway.
    This removes the on-device mask multiply entirely.
  * bf16 everywhere on the wire (p, f, h, W_h, W_a): ~2.9x less HBM
    traffic combined with compaction (42 MiB -> ~14.6 MiB per core).
  * ATT_HID on partitions ("layout 2"): the att_h broadcast-add fuses
    into the ACT tanh as a per-partition bias AP, and the W_a reduction
    becomes PE matmuls (tanh tile stationary [128a x 128l], wa column
    streaming, N=1) that emit scores directly in the [128(l), nch]
    column layout phase B consumes.  Phase A uses zero DVE ops.
  * softmax without max-subtraction (|scores| <= sum|W_a| ~ 11.3, exp
    can't overflow f32), Z via ones-matmul partition reduce.
  * phase B: out[b] = w @ f as PE matmuls, w column [128,1] stationary,
    f tile [128, 512] streaming, accumulated over l-chunks in PSUM.
  * host-side prep is layout/dtype only (transposes, gather by mask,
    bf16 casts): all arithmetic of the module stays on device.
"""

import sys

import ml_dtypes
import numpy as np

sys.path.insert(0, "/opt/trn_rl_repo")

import concourse.bass as bass  # noqa: E402
import concourse.tile as tile  # noqa: E402
from concourse import bacc, mybir  # noqa: E402
from concourse.bass_utils import run_bass_kernel_spmd  # noqa: E402

N_CORES = 8
B, L, RNN, A = 32, 2048, 1024, 512
BS = B // N_CORES          # 4 batch rows per core
NRC = RNN // 128           # 8 contraction chunks for att_h
NAC = A // 128             # 4 a-chunks (ATT_HID on partitions)
NH = RNN // 512            # 2 PSUM halves for phase B

F32 = mybir.dt.float32
BF16 = mybir.dt.bfloat16
TANH = mybir.ActivationFunctionType.Tanh
EXP = mybir.ActivationFunctionType.Exp
IDENT = mybir.ActivationFunctionType.Identity

KERNEL_VERSION = 9


def build_program(nch, bs=BS, rnn=RNN, a=A):
    lc = nch * 128
    nc = bacc.Bacc(None, target_bir_lowering=False)
    # p2[b, q, ac, l] = p_padded[b, l, ac*128+q]   (A on partitions)
    p = nc.dram_tensor("p", [bs, 128, NAC, lc], BF16, kind="ExternalInput")
    # f2[b, q, n, r] = f_padded[b, n*128+q, r]     (L on partitions)
    f = nc.dram_tensor("f", [bs, 128, nch, rnn], BF16, kind="ExternalInput")
    # h2[q, rc, b] = h[b, rc*128+q]
    h2 = nc.dram_tensor("h2", [128, NRC, bs], BF16, kind="ExternalInput")
    # wh2[q, rc, a] = W_h[a, rc*128+q]
    wh2 = nc.dram_tensor("wh2", [128, NRC, a], BF16, kind="ExternalInput")
    # bh2[q, ac] = b_h[ac*128+q],  wa2[q, ac] = W_a[0, ac*128+q]
    bh2 = nc.dram_tensor("bh2", [128, NAC], F32, kind="ExternalInput")
    wa2 = nc.dram_tensor("wa2", [128, NAC], BF16, kind="ExternalInput")
    # unused input whose SHAPE encodes the kernel version: the compile
    # cache keys on the HLO signature (names/shapes), NOT the embedded
    # BIR -- without this, a rebuilt kernel with unchanged I/O silently
    # re-runs the previously cached NEFF.
    ver = nc.dram_tensor("ver", [nch, KERNEL_VERSION], F32,
                         kind="ExternalInput")
    out = nc.dram_tensor("out", [bs, rnn], F32, kind="ExternalOutput")

    with tile.TileContext(nc) as tc:
        with (
            tc.tile_pool(name="singles", bufs=1) as singles,
            tc.tile_pool(name="ppool", bufs=3) as ppool,
            tc.tile_pool(name="fpool", bufs=3) as fpool,
            tc.tile_pool(name="sm", bufs=3) as smpool,
            tc.tile_pool(name="respool", bufs=2) as respool,
            tc.tile_pool(name="ps_sc", bufs=2, space="PSUM") as ps_sc,
            tc.tile_pool(name="psacc", bufs=2, space="PSUM") as psacc,
            tc.tile_pool(name="pssmall", bufs=1, space="PSUM") as pssmall,
        ):
            # ---- constants ----
            wh_sb = singles.tile([128, NRC, a], BF16)
            nc.sync.dma_start(out=wh_sb, in_=wh2[:, :, :])
            h_sb = singles.tile([128, NRC, bs], BF16)
            nc.sync.dma_start(out=h_sb, in_=h2[:, :, :])
            bh_sb = singles.tile([128, NAC], F32)
            nc.sync.dma_start(out=bh_sb, in_=bh2[:, :])
            wa_sb = singles.tile([128, NAC], BF16)
            nc.sync.dma_start(out=wa_sb, in_=wa2[:, :])
            ver_sb = singles.tile([nch, KERNEL_VERSION], F32)
            nc.sync.dma_start(out=ver_sb, in_=ver[:, :])
            ones_sb = singles.tile([128, 1], F32)
            nc.vector.memset(ones_sb, 1.0)
            attb = singles.tile([128, NAC, bs], F32)

            # ---- phase 0: attb[:, ac, b] = (W_h @ h[b] + b_h) in
            # a-on-partitions layout, via PE with bs as the stream dim.
            with tc.tile_pool(name="ps0", bufs=1, space="PSUM") as ps0:
                for ac in range(NAC):
                    ah_ps = ps0.tile([128, bs], F32, tag="ah", name=f"ah{ac}")
                    for rc in range(NRC):
                        nc.tensor.matmul(
                            ah_ps, lhsT=wh_sb[:, rc, ac * 128:(ac + 1) * 128],
                            rhs=h_sb[:, rc, :],
                            start=(rc == 0), stop=(rc == NRC - 1))
                    nc.scalar.activation(out=attb[:, ac, :], in_=ah_ps,
                                         func=IDENT, bias=bh_sb[:, ac:ac + 1])

            for b in range(bs):
                ptile = ppool.tile([128, NAC, lc], BF16, tag="p")
                nc.sync.dma_start(out=ptile, in_=p[b, :, :, :])
                ftile = fpool.tile([128, nch, rnn], BF16, tag="f")
                nc.sync.dma_start(out=ftile, in_=f[b, :, :, :])

                # ---- phase A: tanh with fused per-partition bias, then
                # scores[lch*128+m] via PE (tanh stationary, wa streams).
                for ac in range(NAC):
                    nc.scalar.activation(
                        out=ptile[:, ac, :], in_=ptile[:, ac, :],
                        func=TANH, bias=attb[:, ac, b:b + 1])
                sc_ps = ps_sc.tile([128, nch], F32, tag="sc", name=f"sc{b}")
                for lch in range(nch):
                    for ac in range(NAC):
                        nc.tensor.matmul(
                            sc_ps[:, lch:lch + 1],
                            lhsT=ptile[:, ac, lch * 128:(lch + 1) * 128],
                            rhs=wa_sb[:, ac:ac + 1],
                            start=(ac == 0), stop=(ac == NAC - 1))

                # ---- softmax weights (no max subtraction needed) ----
                w_sb = smpool.tile([128, nch], BF16, tag="w")
                nc.scalar.activation(out=w_sb, in_=sc_ps, func=EXP)
                zpart = smpool.tile([128, 1], F32, tag="zpart")
                nc.vector.reduce_sum(zpart, w_sb, axis=mybir.AxisListType.X)
                z_ps = pssmall.tile([1, 1], F32, tag="zps", name=f"z{b}")
                nc.tensor.matmul(z_ps, lhsT=ones_sb, rhs=zpart,
                                 start=True, stop=True)
                zinv = smpool.tile([1, 1], F32, tag="zinv")
                nc.vector.reciprocal(zinv, z_ps)

                # ---- phase B: out[b] = (w/Z) @ att_feats[b] ----
                rps = [psacc.tile([1, 512], F32, tag=f"r{hh}",
                                  name=f"r{b}_{hh}") for hh in range(NH)]
                for lch in range(nch):
                    for hh in range(NH):
                        nc.tensor.matmul(
                            rps[hh], lhsT=w_sb[:, lch:lch + 1],
                            rhs=ftile[:, lch, hh * 512:(hh + 1) * 512],
                            start=(lch == 0), stop=(lch == nch - 1))
                res = respool.tile([1, rnn], F32, tag="res")
                for hh in range(NH):
                    nc.vector.tensor_scalar_mul(
                        res[:, hh * 512:(hh + 1) * 512], rps[hh], zinv)
                nc.sync.dma_start(out=out[b:b + 1, :], in_=res)
    nc.finalize()
    return nc


_PROGS = {}


def _get_program(nch):
    if nch not in _PROGS:
        _PROGS[nch] = build_program(nch)
    return _PROGS[nch]


def make_in_maps(h, att_feats, p_att_feats, mask, W_h, b_h, W_a):
    h = np.asarray(h, dtype=np.float32)
    att_feats = np.asarray(att_feats, dtype=np.float32)
    p_att_feats = np.asarray(p_att_feats, dtype=np.float32)
    mask = np.asarray(mask)
    W_h = np.asarray(W_h, np.float32)
    b_h = np.asarray(b_h, np.float32).reshape(A)
    wa = np.asarray(W_a, np.float32).reshape(A)

    keep = ~mask                                   # [B, L] kept positions
    cnts = keep.sum(axis=1)
    nch = max(1, -(-int(cnts.max()) // 128))
    lc = nch * 128

    # padding p-row: tanh saturates to -sign(wa) => score = -sum|wa|
    p_pad = np.where(wa >= 0.0, -20.0, 20.0).astype(ml_dtypes.bfloat16)

    p2 = np.empty((B, 128, NAC, lc), dtype=ml_dtypes.bfloat16)
    f2 = np.zeros((B, 128, nch, RNN), dtype=ml_dtypes.bfloat16)
    for b in range(B):
        idx = np.flatnonzero(keep[b])
        n = idx.size
        pb = np.empty((lc, A), dtype=ml_dtypes.bfloat16)
        pb[:n] = p_att_feats[b, idx]
        pb[n:] = p_pad
        # [lc, A] -> [128(q), NAC, lc]
        p2[b] = pb.reshape(lc, NAC, 128).transpose(2, 1, 0)
        fb = np.zeros((lc, RNN), dtype=ml_dtypes.bfloat16)
        fb[:n] = att_feats[b, idx].astype(ml_dtypes.bfloat16)
        # [lc, RNN] -> [128(q), nch, RNN]
        f2[b] = fb.reshape(nch, 128, RNN).transpose(1, 0, 2)

    # h2[q, rc, b] per core;  wh2[q, rc, a];  bh2/wa2 [q, ac]
    wh2 = np.ascontiguousarray(
        W_h.T.reshape(NRC, 128, A).transpose(1, 0, 2)).astype(
            ml_dtypes.bfloat16)
    bh2 = np.ascontiguousarray(b_h.reshape(NAC, 128).T)
    wa2 = np.ascontiguousarray(wa.reshape(NAC, 128).T).astype(
        ml_dtypes.bfloat16)

    ver = np.zeros((nch, KERNEL_VERSION), np.float32)
    in_maps = []
    for c in range(N_CORES):
        s = slice(c * BS, (c + 1) * BS)
        h2c = np.ascontiguousarray(
            h[s].reshape(BS, NRC, 128).transpose(2, 1, 0)).astype(
                ml_dtypes.bfloat16)
        in_maps.append({
            "p": np.ascontiguousarray(p2[s]),
            "f": np.ascontiguousarray(f2[s]),
            "h2": h2c,
            "wh2": wh2,
            "bh2": bh2,
            "wa2": wa2,
            "ver": ver,
        })
    return in_maps, nch


def run_sharded(inputs, trace=False, **kwargs):
    in_maps, nch = make_in_maps(
        inputs["h"], inputs["att_feats"], inputs["p_att_feats"],
        inputs["mask"], inputs["W_h"], inputs["b_h"], inputs["W_a"])
    nc = _get_program(nch)
    return run_bass_kernel_spmd(nc, in_maps, core_ids=list(range(N_CORES)),
                                trace=trace, **kwargs)


def kernel(h, att_feats, p_att_feats, mask, W_h, b_h, W_a, b_a):
    res = run_sharded({
        "h": h, "att_feats": att_feats, "p_att_feats": p_att_feats,
        "mask": mask, "W_h": W_h, "b_h": b_h, "W_a": W_a, "b_a": b_a})
    return np.concatenate([res.results[c]["out"] for c in range(N_CORES)],
                          axis=0).astype(np.float32)


# revision 9
# speedup vs baseline: 3.5647x; 1.1129x over previous
"""Trainium2 Bass kernel for the Attention2 module (sparse attention).

Computation (per batch row b):
    att_h  = h[b] @ W_h.T + b_h                      # [A]
    dot    = tanh(p_att_feats[b] + att_h)            # [L, A]
    scores = dot @ W_a[0]  (+ b_a, dropped: softmax shift-invariant)
    scores = where(mask, -1e8, scores)
    w      = softmax(scores)                         # [L]
    out[b] = w @ att_feats[b]                        # [R]

Sharding: data-parallel over batch B=32 across 8 cores (4 rows/core).

Key optimizations vs the dense baseline:
  * mask-compaction on host: masked positions get softmax weight exactly
    0 in the reference (exp(-1e8) == 0), so their p/f rows never need to
    reach the device.  Rows are gathered to the front and padded to a
    multiple of 128.  Padding p-rows are filled with -sign(W_a)*20 so
    tanh saturates to -sign(W_a) and the padded score is exactly
    -sum|W_a| ~= -11.3 -- its softmax contribution (~e-11 vs real scores
    ~e^0) is < 1e-5 relative, and the padded f-rows are zero anyway.
    This removes the on-device mask multiply entirely.
  * bf16 everywhere on the wire (p, f, h, W_h, W_a): ~2.9x less HBM
    traffic combined with compaction (42 MiB -> ~14.6 MiB per core).
  * ATT_HID on partitions ("layout 2"): the att_h broadcast-add fuses
    into the ACT tanh as a per-partition bias AP, and the W_a reduction
    becomes PE matmuls (tanh tile stationary [128a x 128l], wa column
    streaming, N=1) that emit scores directly in the [128(l), nch]
    column layout phase B consumes.  Phase A uses zero DVE ops.
  * softmax without max-subtraction (|scores| <= sum|W_a| ~ 11.3, exp
    can't overflow f32), Z via ones-matmul partition reduce.
  * phase B: out[b] = w @ f as PE matmuls, w column [128,1] stationary,
    f tile [128, 512] streaming, accumulated over l-chunks in PSUM.
  * DMA ordering: weights + all four p tiles are queued on the sync
    HWDGE ring before the (3x bigger) f tiles, so the tanh->scores chain
    for later rows is never starved behind f traffic.  f tiles arrive in
    3 sub-chunks so phase B overlaps the transfer.  Result DMAs go out
    on the scalar HWDGE ring (independent FIFO).
  * host-side prep is layout/dtype only (transposes, gather by mask,
    bf16 casts): all arithmetic of the module stays on device.
"""

import sys

import ml_dtypes
import numpy as np

sys.path.insert(0, "/opt/trn_rl_repo")

import concourse.bass as bass  # noqa: E402
import concourse.tile as tile  # noqa: E402
from concourse import bacc, mybir  # noqa: E402
from concourse.bass_utils import run_bass_kernel_spmd  # noqa: E402

N_CORES = 8
B, L, RNN, A = 32, 2048, 1024, 512
BS = B // N_CORES          # 4 batch rows per core
NRC = RNN // 128           # 8 contraction chunks for att_h
NAC = A // 128             # 4 a-chunks (ATT_HID on partitions)
NH = RNN // 512            # 2 PSUM halves for phase B
NFC = 3                    # f sub-chunks per row

F32 = mybir.dt.float32
FP8 = mybir.dt.float8e4
BF16 = mybir.dt.bfloat16
TANH = mybir.ActivationFunctionType.Tanh
EXP = mybir.ActivationFunctionType.Exp
IDENT = mybir.ActivationFunctionType.Identity

KERNEL_VERSION = 11


def build_program(nch, bs=BS, rnn=RNN, a=A):
    lc = nch * 128
    # f sub-chunk boundaries (lch indices)
    cuts = [round(i * nch / NFC) for i in range(NFC + 1)]
    nc = bacc.Bacc(None, target_bir_lowering=False)
    # p2[b, q, ac, l] = p_padded[b, l, ac*128+q]   (A on partitions)
    p = nc.dram_tensor("p", [bs, 128, NAC, lc], FP8, kind="ExternalInput")
    # f2[b, q, n, r] = f_padded[b, n*128+q, r]     (L on partitions)
    f = nc.dram_tensor("f", [bs, 128, nch, rnn], BF16, kind="ExternalInput")
    # h2[q, rc, b] = h[b, rc*128+q]
    h2 = nc.dram_tensor("h2", [128, NRC, bs], BF16, kind="ExternalInput")
    # wh2[q, rc, a] = W_h[a, rc*128+q]
    wh2 = nc.dram_tensor("wh2", [128, NRC, a], BF16, kind="ExternalInput")
    # bh2[q, ac] = b_h[ac*128+q],  wa2[q, ac] = W_a[0, ac*128+q]
    bh2 = nc.dram_tensor("bh2", [128, NAC, bs], F32, kind="ExternalInput")
    wa2 = nc.dram_tensor("wa2", [128, NAC], BF16, kind="ExternalInput")
    # unused input whose SHAPE encodes the kernel version: the compile
    # cache keys on the HLO signature (names/shapes), NOT the embedded
    # BIR -- without this, a rebuilt kernel with unchanged I/O silently
    # re-runs the previously cached NEFF.
    ver = nc.dram_tensor("ver", [nch, KERNEL_VERSION], F32,
                         kind="ExternalInput")
    out = nc.dram_tensor("out", [bs, rnn], F32, kind="ExternalOutput")

    with tile.TileContext(nc) as tc:
        with (
            tc.tile_pool(name="singles", bufs=1) as singles,
            tc.tile_pool(name="ppool", bufs=bs) as ppool,
            tc.tile_pool(name="thpool", bufs=bs) as thpool,
            tc.tile_pool(name="fpool", bufs=bs) as fpool,
            tc.tile_pool(name="sm", bufs=4) as smpool,
            tc.tile_pool(name="respool", bufs=2) as respool,
            tc.tile_pool(name="ps_sc", bufs=2, space="PSUM") as ps_sc,
            tc.tile_pool(name="psacc", bufs=2, space="PSUM") as psacc,
            tc.tile_pool(name="pssmall", bufs=1, space="PSUM") as pssmall,
        ):
            # ---- constants (sync ring, ahead of the bulk loads) ----
            wh_sb = singles.tile([128, NRC, a], BF16)
            nc.sync.dma_start(out=wh_sb, in_=wh2[:, :, :])
            h_sb = singles.tile([128, NRC, bs], BF16)
            nc.sync.dma_start(out=h_sb, in_=h2[:, :, :])
            bh_sb = singles.tile([128, NAC, bs], F32)
            nc.sync.dma_start(out=bh_sb, in_=bh2[:, :, :])
            wa_sb = singles.tile([128, NAC], BF16)
            nc.sync.dma_start(out=wa_sb, in_=wa2[:, :])
            ver_sb = singles.tile([nch, KERNEL_VERSION], F32)
            nc.sync.dma_start(out=ver_sb, in_=ver[:, :])
            ones_sb = singles.tile([128, 1], F32)
            nc.vector.memset(ones_sb, 1.0)
            attb = singles.tile([128, NAC, bs], F32)
            # warm the ACT function table while DMAs stream (no data dep)
            warm_sb = singles.tile([128, 1], BF16)
            nc.scalar.activation(out=warm_sb, in_=ones_sb, func=TANH)

            # ---- queue all p tiles, then all f tiles (sync ring FIFO:
            # later rows' p must not sit behind earlier rows' f).
            ptiles, ftiles = [], []
            for b in range(bs):
                ptile = ppool.tile([128, NAC, lc], FP8, tag="p",
                                   name=f"pt{b}")
                nc.sync.dma_start(out=ptile, in_=p[b, :, :, :])
                ptiles.append(ptile)
            for b in range(bs):
                ftiles.append(fpool.tile([128, nch, rnn], BF16, tag="f",
                                         name=f"ft{b}"))
            for b in range(bs):
                rcuts = (cuts if b < bs - 1
                         else list(range(nch + 1)))   # last row: per-lch
                for k in range(len(rcuts) - 1):
                    c0, c1 = rcuts[k], rcuts[k + 1]
                    nc.sync.dma_start(out=ftiles[b][:, c0:c1, :],
                                      in_=f[b, :, c0:c1, :])

            # ---- phase 0: attb[:, ac, b] = (W_h @ h[b] + b_h) in
            # a-on-partitions layout; single PSUM tile, bs as stream dim.
            with tc.tile_pool(name="ps0", bufs=1, space="PSUM") as ps0:
                ah_ps = ps0.tile([128, NAC, bs], F32, tag="ah")
                for ac in range(NAC):
                    for rc in range(NRC):
                        nc.tensor.matmul(
                            ah_ps[:, ac, :],
                            lhsT=wh_sb[:, rc, ac * 128:(ac + 1) * 128],
                            rhs=h_sb[:, rc, :],
                            start=(rc == 0), stop=(rc == NRC - 1))
                # bias add on DVE (idle engine; keeps ACT chain clean)
                nc.vector.tensor_add(attb, ah_ps, bh_sb)

            # ---- pass 1 (phase A for every row): the ACT and PE queues
            # are in-order, so nothing f-gated may be emitted here or
            # later rows' tanh/score work would stall behind it.
            w_sbs, zinvs = [], []
            for b in range(bs):
                ptile = ptiles[b]
                # tanh with fused per-partition bias (fp8 in, bf16 out)
                th = thpool.tile([128, NAC, lc], BF16, tag="th",
                                 name=f"th{b}")
                for ac in range(NAC):
                    nc.scalar.activation(
                        out=th[:, ac, :], in_=ptile[:, ac, :],
                        func=TANH, bias=attb[:, ac, b:b + 1])
                # scores: lch-outer so each PSUM column's accumulation
                # group is issued contiguously (interleaved groups in one
                # bank corrupt accumulation on HW)
                sc_ps = ps_sc.tile([128, nch], F32, tag="sc", name=f"sc{b}")
                for lch in range(nch):
                    for ac in range(NAC):
                        nc.tensor.matmul(
                            sc_ps[:, lch:lch + 1],
                            lhsT=th[:, ac, lch * 128:(lch + 1) * 128],
                            rhs=wa_sb[:, ac:ac + 1],
                            start=(ac == 0), stop=(ac == NAC - 1))

                # softmax weights (no max subtraction needed)
                w_sb = smpool.tile([128, nch], BF16, tag="w", name=f"w{b}")
                nc.scalar.activation(out=w_sb, in_=sc_ps, func=EXP)
                zpart = smpool.tile([128, 1], F32, tag="zpart",
                                    name=f"zp{b}")
                nc.vector.reduce_sum(zpart, w_sb, axis=mybir.AxisListType.X)
                z_ps = pssmall.tile([1, 1], F32, tag="zps", name=f"z{b}")
                nc.tensor.matmul(z_ps, lhsT=ones_sb, rhs=zpart,
                                 start=True, stop=True)
                zinv = smpool.tile([1, 1], F32, tag="zinv", name=f"zi{b}")
                nc.vector.reciprocal(zinv, z_ps)
                w_sbs.append(w_sb)
                zinvs.append(zinv)

            # ---- pass 2 (phase B): out[b] = (w/Z) @ att_feats[b]; the
            # matmuls for each f sub-chunk start as its transfer lands.
            for b in range(bs):
                ftile, w_sb, zinv = ftiles[b], w_sbs[b], zinvs[b]
                rps = [psacc.tile([1, 512], F32, tag=f"r{hh}",
                                  name=f"r{b}_{hh}") for hh in range(NH)]
                for lch in range(nch):
                    for hh in range(NH):
                        nc.tensor.matmul(
                            rps[hh], lhsT=w_sb[:, lch:lch + 1],
                            rhs=ftile[:, lch, hh * 512:(hh + 1) * 512],
                            start=(lch == 0), stop=(lch == nch - 1))
                res = respool.tile([1, rnn], F32, tag="res", name=f"res{b}")
                for hh in range(NH):
                    nc.vector.tensor_scalar_mul(
                        res[:, hh * 512:(hh + 1) * 512], rps[hh], zinv)
                # out triggers sit on the sync ring AFTER every input
                # load trigger, so they block nothing.
                nc.sync.dma_start(out=out[b:b + 1, :], in_=res)
    nc.finalize()
    return nc


_PROGS = {}


def _get_program(nch):
    if nch not in _PROGS:
        _PROGS[nch] = build_program(nch)
    return _PROGS[nch]


def make_in_maps(h, att_feats, p_att_feats, mask, W_h, b_h, W_a):
    h = np.asarray(h, dtype=np.float32)
    att_feats = np.asarray(att_feats, dtype=np.float32)
    p_att_feats = np.asarray(p_att_feats, dtype=np.float32)
    mask = np.asarray(mask)
    W_h = np.asarray(W_h, np.float32)
    b_h = np.asarray(b_h, np.float32).reshape(A)
    wa = np.asarray(W_a, np.float32).reshape(A)

    keep = ~mask                                   # [B, L] kept positions
    cnts = keep.sum(axis=1)
    nch = max(1, -(-int(cnts.max()) // 128))
    lc = nch * 128

    # padding p-row: tanh saturates to -sign(wa) => score = -sum|wa|
    p_pad = np.where(wa >= 0.0, -20.0, 20.0).astype(ml_dtypes.float8_e4m3)

    p2 = np.empty((B, 128, NAC, lc), dtype=ml_dtypes.float8_e4m3)
    f2 = np.zeros((B, 128, nch, RNN), dtype=ml_dtypes.bfloat16)
    for b in range(B):
        idx = np.flatnonzero(keep[b])
        n = idx.size
        pb = np.empty((lc, A), dtype=ml_dtypes.float8_e4m3)
        pb[:n] = p_att_feats[b, idx]
        pb[n:] = p_pad
        # [lc, A] -> [128(q), NAC, lc]
        p2[b] = pb.reshape(lc, NAC, 128).transpose(2, 1, 0)
        fb = np.zeros((lc, RNN), dtype=ml_dtypes.bfloat16)
        fb[:n] = att_feats[b, idx].astype(ml_dtypes.bfloat16)
        # [lc, RNN] -> [128(q), nch, RNN]
        f2[b] = fb.reshape(nch, 128, RNN).transpose(1, 0, 2)

    # h2[q, rc, b] per core;  wh2[q, rc, a];  bh2/wa2 [q, ac]
    wh2 = np.ascontiguousarray(
        W_h.T.reshape(NRC, 128, A).transpose(1, 0, 2)).astype(
            ml_dtypes.bfloat16)
    bh2 = np.ascontiguousarray(np.broadcast_to(
        b_h.reshape(NAC, 128).T[:, :, None], (128, NAC, BS)))
    wa2 = np.ascontiguousarray(wa.reshape(NAC, 128).T).astype(
        ml_dtypes.bfloat16)

    ver = np.zeros((nch, KERNEL_VERSION), np.float32)
    in_maps = []
    for c in range(N_CORES):
        s = slice(c * BS, (c + 1) * BS)
        h2c = np.ascontiguousarray(
            h[s].reshape(BS, NRC, 128).transpose(2, 1, 0)).astype(
                ml_dtypes.bfloat16)
        in_maps.append({
            "p": np.ascontiguousarray(p2[s]),
            "f": np.ascontiguousarray(f2[s]),
            "h2": h2c,
            "wh2": wh2,
            "bh2": bh2,
            "wa2": wa2,
            "ver": ver,
        })
    return in_maps, nch


def run_sharded(inputs, trace=False, **kwargs):
    in_maps, nch = make_in_maps(
        inputs["h"], inputs["att_feats"], inputs["p_att_feats"],
        inputs["mask"], inputs["W_h"], inputs["b_h"], inputs["W_a"])
    nc = _get_program(nch)
    return run_bass_kernel_spmd(nc, in_maps, core_ids=list(range(N_CORES)),
                                trace=trace, **kwargs)


def kernel(h, att_feats, p_att_feats, mask, W_h, b_h, W_a, b_a):
    res = run_sharded({
        "h": h, "att_feats": att_feats, "p_att_feats": p_att_feats,
        "mask": mask, "W_h": W_h, "b_h": b_h, "W_a": W_a, "b_a": b_a})
    return np.concatenate([res.results[c]["out"] for c in range(N_CORES)],
                          axis=0).astype(np.float32)


# revision 10
# speedup vs baseline: 3.6815x; 1.0328x over previous
"""Trainium2 Bass kernel for the Attention2 module (sparse attention).

Computation (per batch row b):
    att_h  = h[b] @ W_h.T + b_h                      # [A]
    dot    = tanh(p_att_feats[b] + att_h)            # [L, A]
    scores = dot @ W_a[0]  (+ b_a, dropped: softmax shift-invariant)
    scores = where(mask, -1e8, scores)
    w      = softmax(scores)                         # [L]
    out[b] = w @ att_feats[b]                        # [R]

Sharding: data-parallel over batch B=32 across 8 cores (4 rows/core).

Key optimizations vs the dense baseline:
  * mask-compaction on host: masked positions get softmax weight exactly
    0 in the reference (exp(-1e8) == 0), so their p/f rows never need to
    reach the device.  Rows are gathered to the front and padded to a
    multiple of 128.  Padding p-rows are filled with -sign(W_a)*20 so
    tanh saturates to -sign(W_a) and the padded score is exactly
    -sum|W_a| ~= -11.3 -- its softmax contribution (~e-11 vs real scores
    ~e^0) is < 1e-5 relative, and the padded f-rows are zero anyway.
    This removes the on-device mask multiply entirely.
  * bf16 everywhere on the wire (p, f, h, W_h, W_a): ~2.9x less HBM
    traffic combined with compaction (42 MiB -> ~14.6 MiB per core).
  * ATT_HID on partitions ("layout 2"): the att_h broadcast-add fuses
    into the ACT tanh as a per-partition bias AP, and the W_a reduction
    becomes PE matmuls (tanh tile stationary [128a x 128l], wa column
    streaming, N=1) that emit scores directly in the [128(l), nch]
    column layout phase B consumes.  Phase A uses zero DVE ops.
  * softmax without max-subtraction (|scores| <= sum|W_a| ~ 11.3, exp
    can't overflow f32), Z via ones-matmul partition reduce.
  * phase B: out[b] = w @ f as PE matmuls, w column [128,1] stationary,
    f tile [128, 512] streaming, accumulated over l-chunks in PSUM.
  * DMA ordering: weights + all four p tiles are queued on the sync
    HWDGE ring before the (3x bigger) f tiles, so the tanh->scores chain
    for later rows is never starved behind f traffic.  f tiles arrive in
    3 sub-chunks so phase B overlaps the transfer.  Result DMAs go out
    on the scalar HWDGE ring (independent FIFO).
  * host-side prep is layout/dtype only (transposes, gather by mask,
    bf16 casts): all arithmetic of the module stays on device.
"""

import sys

import ml_dtypes
import numpy as np

sys.path.insert(0, "/opt/trn_rl_repo")

import concourse.bass as bass  # noqa: E402
import concourse.tile as tile  # noqa: E402
from concourse import bacc, mybir  # noqa: E402
from concourse.bass_utils import run_bass_kernel_spmd  # noqa: E402

N_CORES = 8
B, L, RNN, A = 32, 2048, 1024, 512
BS = B // N_CORES          # 4 batch rows per core
NRC = RNN // 128           # 8 contraction chunks for att_h
NAC = A // 128             # 4 a-chunks (ATT_HID on partitions)
NH = RNN // 512            # 2 PSUM halves for phase B
NFC = 3                    # f sub-chunks per row

F32 = mybir.dt.float32
FP8 = mybir.dt.float8e4
BF16 = mybir.dt.bfloat16
TANH = mybir.ActivationFunctionType.Tanh
EXP = mybir.ActivationFunctionType.Exp
IDENT = mybir.ActivationFunctionType.Identity

KERNEL_VERSION = 12


def build_program(nch, bs=BS, rnn=RNN, a=A):
    lc = nch * 128
    # f sub-chunk boundaries (lch indices)
    cuts = [round(i * nch / NFC) for i in range(NFC + 1)]
    nc = bacc.Bacc(None, target_bir_lowering=False)
    # p2[b, q, ac, l] = p_padded[b, l, ac*128+q]   (A on partitions)
    p = nc.dram_tensor("p", [bs, 128, NAC, lc], FP8, kind="ExternalInput")
    # f2[b, q, n, r] = f_padded[b, n*128+q, r]     (L on partitions)
    f = nc.dram_tensor("f", [bs, 128, nch, rnn], BF16, kind="ExternalInput")
    # h2[q, rc, b] = h[b, rc*128+q]
    h2 = nc.dram_tensor("h2", [128, NRC, bs], FP8, kind="ExternalInput")
    # wh2[q, rc, a] = W_h[a, rc*128+q]
    wh2 = nc.dram_tensor("wh2", [128, NRC, a], FP8, kind="ExternalInput")
    # bh2[q, ac] = b_h[ac*128+q],  wa2[q, ac] = W_a[0, ac*128+q]
    bh2 = nc.dram_tensor("bh2", [128, NAC, bs], F32, kind="ExternalInput")
    wa2 = nc.dram_tensor("wa2", [128, NAC], BF16, kind="ExternalInput")
    # unused input whose SHAPE encodes the kernel version: the compile
    # cache keys on the HLO signature (names/shapes), NOT the embedded
    # BIR -- without this, a rebuilt kernel with unchanged I/O silently
    # re-runs the previously cached NEFF.
    ver = nc.dram_tensor("ver", [nch, KERNEL_VERSION], F32,
                         kind="ExternalInput")
    out = nc.dram_tensor("out", [bs, rnn], F32, kind="ExternalOutput")

    with tile.TileContext(nc) as tc:
        with (
            tc.tile_pool(name="singles", bufs=1) as singles,
            tc.tile_pool(name="ppool", bufs=bs) as ppool,
            tc.tile_pool(name="thpool", bufs=bs) as thpool,
            tc.tile_pool(name="fpool", bufs=bs) as fpool,
            tc.tile_pool(name="sm", bufs=4) as smpool,
            tc.tile_pool(name="respool", bufs=2) as respool,
            tc.tile_pool(name="ps_sc", bufs=2, space="PSUM") as ps_sc,
            tc.tile_pool(name="psacc", bufs=2, space="PSUM") as psacc,
            tc.tile_pool(name="pssmall", bufs=1, space="PSUM") as pssmall,
        ):
            # ---- constants (sync ring, ahead of the bulk loads;
            # wh/h2/p0 first -- they gate the first tanh) ----
            wh_sb = singles.tile([128, NRC, a], FP8)
            nc.sync.dma_start(out=wh_sb, in_=wh2[:, :, :])
            h_sb = singles.tile([128, NRC, bs], FP8)
            nc.sync.dma_start(out=h_sb, in_=h2[:, :, :])
            ones_sb = singles.tile([128, 1], F32)
            nc.vector.memset(ones_sb, 1.0)
            attb = singles.tile([128, NAC, bs], F32)
            # warm the ACT function table while DMAs stream (no data dep)
            warm_sb = singles.tile([128, 1], BF16)
            nc.scalar.activation(out=warm_sb, in_=ones_sb, func=TANH)

            # ---- queue all p tiles, then all f tiles (sync ring FIFO:
            # later rows' p must not sit behind earlier rows' f).
            ptiles, ftiles = [], []
            for b in range(bs):
                ptile = ppool.tile([128, NAC, lc], FP8, tag="p",
                                   name=f"pt{b}")
                nc.sync.dma_start(out=ptile, in_=p[b, :, :, :])
                ptiles.append(ptile)
                if b == 0:
                    bh_sb = singles.tile([128, NAC, bs], F32)
                    nc.sync.dma_start(out=bh_sb, in_=bh2[:, :, :])
                    wa_sb = singles.tile([128, NAC], BF16)
                    nc.sync.dma_start(out=wa_sb, in_=wa2[:, :])
            ver_sb = singles.tile([nch, KERNEL_VERSION], F32)
            nc.sync.dma_start(out=ver_sb, in_=ver[:, :])
            for b in range(bs):
                ftiles.append(fpool.tile([128, nch, rnn], BF16, tag="f",
                                         name=f"ft{b}"))
            for b in range(bs):
                rcuts = (cuts if b < bs - 1
                         else list(range(nch + 1)))   # last row: per-lch
                for k in range(len(rcuts) - 1):
                    c0, c1 = rcuts[k], rcuts[k + 1]
                    nc.sync.dma_start(out=ftiles[b][:, c0:c1, :],
                                      in_=f[b, :, c0:c1, :])

            # ---- phase 0: attb[:, ac, b] = (W_h @ h[b] + b_h) in
            # a-on-partitions layout; single PSUM tile, bs as stream dim.
            with tc.tile_pool(name="ps0", bufs=1, space="PSUM") as ps0:
                ah_ps = ps0.tile([128, NAC, bs], F32, tag="ah")
                for ac in range(NAC):
                    for rc in range(NRC):
                        nc.tensor.matmul(
                            ah_ps[:, ac, :],
                            lhsT=wh_sb[:, rc, ac * 128:(ac + 1) * 128],
                            rhs=h_sb[:, rc, :],
                            start=(rc == 0), stop=(rc == NRC - 1))
                # bias add on DVE (idle engine; keeps ACT chain clean)
                nc.vector.tensor_add(attb, ah_ps, bh_sb)

            # ---- pass 1 (phase A for every row): the ACT and PE queues
            # are in-order, so nothing f-gated may be emitted here or
            # later rows' tanh/score work would stall behind it.
            w_sbs, zinvs = [], []
            for b in range(bs):
                ptile = ptiles[b]
                # tanh with fused per-partition bias (fp8 in, bf16 out)
                th = thpool.tile([128, NAC, lc], BF16, tag="th",
                                 name=f"th{b}")
                for ac in range(NAC):
                    nc.scalar.activation(
                        out=th[:, ac, :], in_=ptile[:, ac, :],
                        func=TANH, bias=attb[:, ac, b:b + 1])
                # scores: lch-outer so each PSUM column's accumulation
                # group is issued contiguously (interleaved groups in one
                # bank corrupt accumulation on HW)
                sc_ps = ps_sc.tile([128, nch], F32, tag="sc", name=f"sc{b}")
                for lch in range(nch):
                    for ac in range(NAC):
                        nc.tensor.matmul(
                            sc_ps[:, lch:lch + 1],
                            lhsT=th[:, ac, lch * 128:(lch + 1) * 128],
                            rhs=wa_sb[:, ac:ac + 1],
                            start=(ac == 0), stop=(ac == NAC - 1))

                # softmax weights (no max subtraction needed)
                w_sb = smpool.tile([128, nch], BF16, tag="w", name=f"w{b}")
                nc.scalar.activation(out=w_sb, in_=sc_ps, func=EXP)
                zpart = smpool.tile([128, 1], F32, tag="zpart",
                                    name=f"zp{b}")
                nc.vector.reduce_sum(zpart, w_sb, axis=mybir.AxisListType.X)
                z_ps = pssmall.tile([1, 1], F32, tag="zps", name=f"z{b}")
                nc.tensor.matmul(z_ps, lhsT=ones_sb, rhs=zpart,
                                 start=True, stop=True)
                zinv = smpool.tile([1, 1], F32, tag="zinv", name=f"zi{b}")
                nc.vector.reciprocal(zinv, z_ps)
                w_sbs.append(w_sb)
                zinvs.append(zinv)

            # ---- pass 2 (phase B): out[b] = (w/Z) @ att_feats[b]; the
            # matmuls for each f sub-chunk start as its transfer lands.
            for b in range(bs):
                ftile, w_sb, zinv = ftiles[b], w_sbs[b], zinvs[b]
                rps = [psacc.tile([1, 512], F32, tag=f"r{hh}",
                                  name=f"r{b}_{hh}") for hh in range(NH)]
                for lch in range(nch):
                    for hh in range(NH):
                        nc.tensor.matmul(
                            rps[hh], lhsT=w_sb[:, lch:lch + 1],
                            rhs=ftile[:, lch, hh * 512:(hh + 1) * 512],
                            start=(lch == 0), stop=(lch == nch - 1))
                res = respool.tile([1, rnn], F32, tag="res", name=f"res{b}")
                for hh in range(NH):
                    nc.vector.tensor_scalar_mul(
                        res[:, hh * 512:(hh + 1) * 512], rps[hh], zinv)
                    # ship each half as soon as it is scaled; out triggers
                    # sit on the sync ring AFTER every input load trigger,
                    # so they block nothing.
                    nc.sync.dma_start(
                        out=out[b:b + 1, hh * 512:(hh + 1) * 512],
                        in_=res[:, hh * 512:(hh + 1) * 512])
    nc.finalize()
    return nc


_PROGS = {}


def _get_program(nch):
    if nch not in _PROGS:
        _PROGS[nch] = build_program(nch)
    return _PROGS[nch]


def make_in_maps(h, att_feats, p_att_feats, mask, W_h, b_h, W_a):
    h = np.asarray(h, dtype=np.float32)
    att_feats = np.asarray(att_feats, dtype=np.float32)
    p_att_feats = np.asarray(p_att_feats, dtype=np.float32)
    mask = np.asarray(mask)
    W_h = np.asarray(W_h, np.float32)
    b_h = np.asarray(b_h, np.float32).reshape(A)
    wa = np.asarray(W_a, np.float32).reshape(A)

    keep = ~mask                                   # [B, L] kept positions
    cnts = keep.sum(axis=1)
    nch = max(1, -(-int(cnts.max()) // 128))
    lc = nch * 128

    # padding p-row: tanh saturates to -sign(wa) => score = -sum|wa|
    p_pad = np.where(wa >= 0.0, -20.0, 20.0).astype(ml_dtypes.float8_e4m3)

    p2 = np.empty((B, 128, NAC, lc), dtype=ml_dtypes.float8_e4m3)
    f2 = np.zeros((B, 128, nch, RNN), dtype=ml_dtypes.bfloat16)
    for b in range(B):
        idx = np.flatnonzero(keep[b])
        n = idx.size
        pb = np.empty((lc, A), dtype=ml_dtypes.float8_e4m3)
        pb[:n] = p_att_feats[b, idx]
        pb[n:] = p_pad
        # [lc, A] -> [128(q), NAC, lc]
        p2[b] = pb.reshape(lc, NAC, 128).transpose(2, 1, 0)
        fb = np.zeros((lc, RNN), dtype=ml_dtypes.bfloat16)
        fb[:n] = att_feats[b, idx].astype(ml_dtypes.bfloat16)
        # [lc, RNN] -> [128(q), nch, RNN]
        f2[b] = fb.reshape(nch, 128, RNN).transpose(1, 0, 2)

    # h2[q, rc, b] per core;  wh2[q, rc, a];  bh2/wa2 [q, ac]
    wh2 = np.ascontiguousarray(
        W_h.T.reshape(NRC, 128, A).transpose(1, 0, 2)).astype(
            ml_dtypes.float8_e4m3)
    bh2 = np.ascontiguousarray(np.broadcast_to(
        b_h.reshape(NAC, 128).T[:, :, None], (128, NAC, BS)))
    wa2 = np.ascontiguousarray(wa.reshape(NAC, 128).T).astype(
        ml_dtypes.bfloat16)

    ver = np.zeros((nch, KERNEL_VERSION), np.float32)
    in_maps = []
    for c in range(N_CORES):
        s = slice(c * BS, (c + 1) * BS)
        h2c = np.ascontiguousarray(
            h[s].reshape(BS, NRC, 128).transpose(2, 1, 0)).astype(
                ml_dtypes.float8_e4m3)
        in_maps.append({
            "p": np.ascontiguousarray(p2[s]),
            "f": np.ascontiguousarray(f2[s]),
            "h2": h2c,
            "wh2": wh2,
            "bh2": bh2,
            "wa2": wa2,
            "ver": ver,
        })
    return in_maps, nch


def run_sharded(inputs, trace=False, **kwargs):
    in_maps, nch = make_in_maps(
        inputs["h"], inputs["att_feats"], inputs["p_att_feats"],
        inputs["mask"], inputs["W_h"], inputs["b_h"], inputs["W_a"])
    nc = _get_program(nch)
    return run_bass_kernel_spmd(nc, in_maps, core_ids=list(range(N_CORES)),
                                trace=trace, **kwargs)


def kernel(h, att_feats, p_att_feats, mask, W_h, b_h, W_a, b_a):
    res = run_sharded({
        "h": h, "att_feats": att_feats, "p_att_feats": p_att_feats,
        "mask": mask, "W_h": W_h, "b_h": b_h, "W_a": W_a, "b_a": b_a})
    return np.concatenate([res.results[c]["out"] for c in range(N_CORES)],
                          axis=0).astype(np.float32)
